# revision 20
# baseline (speedup 1.0000x reference)
"""Trainium2 Bass kernel for AllExamplesPairwiseMSELossAndBCEWithLogitsLoss.

loss = BCE_with_logits(pred, psi) + 10 * sum_valid((e_i - e_j)^2) / n_valid
where e = pred - logit(psi) and valid pairs satisfy |psi_i - psi_j| >= 0.05.

Uses the identity pred_diff - true_diff = e_i - e_j and the decomposition
  S = sum_{r,j} M_rj (e_r - e_j)^2 = sum_j [C_j e_j^2 + Q_j - 2 e_j P_j]
with (C,P,Q)_j = sum_r M_rj * (1, e_r, e_r^2), so the only O(N^2) work is
mask generation (VectorEngine dual-op tensor_scalar, fp32 compare -> bf16
mask) and the row contraction (TensorEngine matmul, bf16 masks streamed
against a tiny stationary [128,5] operand (1, e_hi, e_lo, e2_hi, e2_lo) --
hi/lo bf16 splitting keeps ~1e-5 precision at 1 cycle/column).

Data-parallel over 8 NeuronCores: core c owns rows [1024c, 1024c+1024).
Each core emits C/P/Q [5, 8192] + per-partition BCE partials; the host
unshards by summing per-core partials into the final scalar.
"""

import numpy as np

import concourse.bass as bass
import concourse.bacc as bacc
import concourse.tile as tile
import concourse.mybir as mybir
from concourse.bass_utils import run_bass_kernel_spmd

F32 = mybir.dt.float32
BF16 = mybir.dt.bfloat16
ALU = mybir.AluOpType
ACT = mybir.ActivationFunctionType

N = 8192
NCORES = 8
SLAB = N // NCORES          # 1024 rows per core
RB = SLAB // 128            # 8 row blocks of 128
W = 512                     # column window (one PSUM bank of fp32)
# circulant column window: core c covers columns (1024c + k) mod N, k < NW.
# Each unordered pair lands in exactly one core's window, except the own-slab
# (k < 1024) and antipode (k >= 4096) regions which pair up across cores --
# region coefficients (1, 2, 1) in the host reduction count every ordered
# pair exactly once.
NW = 5 * SLAB               # 5120 columns per core
CHUNKS = (2560, 2560)
CHUNK = 2560                # max DVE mask tile width
THRESH = 0.05
EPS = 1e-7
MSE_WEIGHT = 10.0

# (block, half) mask units computed on the ScalarEngine via Sign instead of
# the VectorEngine's is_ge/is_le. For those, mask_half = (sign +- 1)/2, which
# is folded in as (+-w5/2)^T sign plus a per-column-constant correction of
# 0.5 * sum_{r in block} w5[r] that the host adds back (see kernel()).
ACT_SET = ((0, 1), (2, 1), (4, 1), (6, 1), (7, 0))

_compiled_nc = None


def build_nc():
    nc = bacc.Bacc("TRN2", target_bir_lowering=False, debug=False, num_devices=NCORES)

    cw_d = nc.dram_tensor("cw", [1, NW], F32, kind="ExternalInput")      # psi, column order
    cs_d = nc.dram_tensor("cs", [128, RB], F32, kind="ExternalInput")   # psi slab, [p, b] = psi[128b + p]
    ps_d = nc.dram_tensor("ps", [128, RB], F32, kind="ExternalInput")   # pred slab, same layout
    out_d = nc.dram_tensor("out", [6, NW], F32, kind="ExternalOutput")

    with tile.TileContext(nc) as tc:
        with (
            tc.tile_pool(name="singles", bufs=1) as singles,
            tc.tile_pool(name="small", bufs=1) as small,
            tc.tile_pool(name="mdve", bufs=14) as mdve,
            tc.tile_pool(name="mact", bufs=7) as mact,
            tc.tile_pool(name="drains", bufs=2) as drains,
            tc.tile_pool(name="psum", bufs=1, space="PSUM") as psum_pool,
        ):
            # slab inputs first (tiny, and the whole prep chain hangs off them),
            # then the column broadcasts fill cj while prep runs
            cs = singles.tile([128, RB], F32)
            ps = singles.tile([128, RB], F32)
            nc.sync.dma_start(cs[:], cs_d[:])
            nc.sync.dma_start(ps[:], ps_d[:])
            cj = singles.tile([128, NW], F32)
            for w in range(NW // W):
                nc.sync.dma_start(
                    cj[:, W * w:W * (w + 1)],
                    cw_d[0:1, W * w:W * (w + 1)].partition_broadcast(128),
                )

            cc = small.tile([128, RB], F32, tag="cc")
            nc.vector.tensor_scalar(
                out=cc[:], in0=cs[:], scalar1=EPS, scalar2=1.0 - EPS,
                op0=ALU.max, op1=ALU.min,
            )
            lnp = small.tile([128, RB], F32, tag="lnp")
            nc.scalar.activation(out=lnp[:], in_=cc[:], func=ACT.Ln)
            ln1mp = small.tile([128, RB], F32, tag="ln1mp")
            nc.scalar.activation(out=ln1mp[:], in_=cc[:], func=ACT.Ln, scale=-1.0, bias=1.0)
            blog = small.tile([128, RB], F32, tag="blog")
            nc.vector.tensor_sub(blog[:], lnp[:], ln1mp[:])
            e_sl = singles.tile([128, RB], F32)
            nc.vector.tensor_sub(e_sl[:], ps[:], blog[:])
            esq_sl = singles.tile([128, RB], F32)
            nc.vector.tensor_mul(esq_sl[:], e_sl[:], e_sl[:])

            # hi/lo bf16 split of e and e^2
            w5 = singles.tile([128, RB, 5], BF16)
            nc.vector.memset(w5[:, :, 0], 1.0)
            nc.vector.tensor_copy(w5[:, :, 1], e_sl[:])          # e_hi (bf16 round)
            e_hi = small.tile([128, RB], F32, tag="ehi")
            nc.vector.tensor_copy(e_hi[:], w5[:, :, 1])          # upcast back
            e_lo = small.tile([128, RB], F32, tag="elo")
            nc.vector.tensor_sub(e_lo[:], e_sl[:], e_hi[:])
            nc.vector.tensor_copy(w5[:, :, 2], e_lo[:])
            nc.vector.tensor_copy(w5[:, :, 3], esq_sl[:])
            q_hi = small.tile([128, RB], F32, tag="qhi")
            nc.vector.tensor_copy(q_hi[:], w5[:, :, 3])
            q_lo = small.tile([128, RB], F32, tag="qlo")
            nc.vector.tensor_sub(q_lo[:], esq_sl[:], q_hi[:])
            nc.vector.tensor_copy(w5[:, :, 4], q_lo[:])

            # half-scaled weights for the Sign-mask units, and their biases
            w5h = singles.tile([128, RB, 5], BF16)
            nc.scalar.activation(out=w5h[:], in_=w5[:], func=ACT.Copy, scale=0.5)
            w5hn = singles.tile([128, RB, 5], BF16)
            nc.scalar.activation(out=w5hn[:], in_=w5[:], func=ACT.Copy, scale=-0.5)
            bp = singles.tile([128, RB], F32)   # -(cs + t), bias for h=0 sign masks
            nc.vector.tensor_scalar(out=bp[:], in0=cs[:], scalar1=-1.0, scalar2=-THRESH,
                                    op0=ALU.mult, op1=ALU.add)
            bm = singles.tile([128, RB], F32)   # -(cs - t), bias for h=1 sign masks
            nc.vector.tensor_scalar(out=bm[:], in0=cs[:], scalar1=-1.0, scalar2=THRESH,
                                    op0=ALU.mult, op1=ALU.add)

            # BCE partial: max(x,0) - x*y + softplus(-|x|), summed along free dim
            relux = small.tile([128, RB], F32, tag="relux")
            nc.vector.tensor_scalar_max(relux[:], ps[:], 0.0)
            xy = small.tile([128, RB], F32, tag="xy")
            nc.vector.tensor_mul(xy[:], ps[:], cs[:])
            t1 = small.tile([128, RB], F32, tag="t1")
            nc.vector.tensor_sub(t1[:], relux[:], xy[:])
            ax = small.tile([128, RB], F32, tag="ax")
            nc.scalar.activation(out=ax[:], in_=ps[:], func=ACT.Abs)
            # softplus(-|x|) = ln(1 + exp(-|x|)); exp(-|x|) in (0.01, 1] here so
            # plain ln(1+t) is accurate (Softplus has no activation table on gen3)
            ex = small.tile([128, RB], F32, tag="ex")
            nc.scalar.activation(out=ex[:], in_=ax[:], func=ACT.Exp, scale=-1.0)
            sp = small.tile([128, RB], F32, tag="sp")
            nc.scalar.activation(out=sp[:], in_=ex[:], func=ACT.Ln, bias=1.0)
            tot = small.tile([128, RB], F32, tag="tot")
            nc.vector.tensor_add(tot[:], t1[:], sp[:])
            bce_red = singles.tile([128, 1], F32)
            nc.vector.tensor_reduce(bce_red[:], tot[:], axis=mybir.AxisListType.X, op=ALU.add)
            nc.sync.dma_start(out_d[5:6, 0:128], bce_red[:])

            # ---- main O(N^2) loop: masks on DVE + ACT(Sign), contraction on PE ----
            c0 = 0
            for chunk in CHUNKS:
                mts = []
                for b in range(RB):
                    for h in range(2):
                        if (b, h) in ACT_SET:
                            mt = mact.tile([128, CHUNK], BF16, tag="ms", name="mts")[:, 0:chunk]
                            nc.scalar.activation(
                                out=mt[:], in_=cj[:, c0:c0 + chunk], func=ACT.Sign,
                                bias=(bp if h == 0 else bm)[:, b:b + 1],
                            )
                            lhs = (w5h if h == 0 else w5hn)[:, b, :]
                        else:
                            mt = mdve.tile([128, CHUNK], BF16, tag="m", name="mtv")[:, 0:chunk]
                            nc.vector.tensor_scalar(
                                out=mt[:], in0=cj[:, c0:c0 + chunk],
                                scalar1=cs[:, b:b + 1],
                                scalar2=THRESH if h == 0 else -THRESH,
                                op0=ALU.subtract,
                                op1=ALU.is_ge if h == 0 else ALU.is_le,
                            )
                            lhs = w5[:, b, :]
                        mts.append((lhs, mt))
                # col-tiled contraction: window w runs in PE column group w%4 and
                # accumulates into its own PSUM bank (concurrent col-tiled matmuls
                # corrupt results when they share a bank; distinct banks are exact)
                ps_t = psum_pool.tile([128, CHUNK], F32, tag="ps", name="pst")[:, 0:chunk]
                for i, (lhs, mt) in enumerate(mts):
                    for w in range(chunk // W):
                        j = w % 4
                        nc.tensor.matmul(
                            ps_t[32 * j:32 * j + 5, W * w:W * (w + 1)],
                            lhs, mt[:, W * w:W * (w + 1)],
                            start=(i == 0), stop=(i == len(mts) - 1),
                            tile_position=(0, 32 * j),
                        )
                for w in range(chunk // W):
                    j = w % 4
                    dr = drains.tile([5, W], F32, tag="drain", name="drt")
                    nc.scalar.copy(dr[:], ps_t[32 * j:32 * j + 5, W * w:W * (w + 1)])
                    nc.sync.dma_start(out_d[0:5, c0 + W * w:c0 + W * (w + 1)], dr[:])
                c0 += chunk

    nc.compile()
    return nc


def _get_nc():
    global _compiled_nc
    if _compiled_nc is None:
        _compiled_nc = build_nc()
    return _compiled_nc


def _logit_np(p):
    p = np.clip(p.astype(np.float32), EPS, np.float32(1.0 - EPS))
    return (np.log(p) - np.log1p(-p)).astype(np.float32)


def run(pred, psi, trace=False):
    """Run the device kernel; returns BassKernelResults."""
    nc = _get_nc()
    in_maps = []
    for c in range(NCORES):
        sl = slice(SLAB * c, SLAB * (c + 1))
        cols = (SLAB * c + np.arange(NW)) % N
        in_maps.append({
            "cw": np.ascontiguousarray(psi[cols].reshape(1, NW)),
            "cs": np.ascontiguousarray(psi[sl].reshape(RB, 128).T),
            "ps": np.ascontiguousarray(pred[sl].reshape(RB, 128).T),
        })
    res = run_bass_kernel_spmd(nc, in_maps, core_ids=list(range(NCORES)), trace=trace)
    return res


def kernel(pred_psi_val, psi_val, use_BCE_loss_only=0):
    pred = np.asarray(pred_psi_val, dtype=np.float32).reshape(N)
    psi = np.asarray(psi_val, dtype=np.float32).reshape(N)

    res = run(pred, psi)

    e = (pred - _logit_np(psi)).astype(np.float64)
    coef = np.ones(NW)
    coef[SLAB:4 * SLAB] = 2.0
    S = 0.0
    n = 0.0
    bce_sum = 0.0
    for c in range(NCORES):
        o = res.results[c]["out"].astype(np.float64)
        es = e[SLAB * c:SLAB * (c + 1)]
        corrP = 0.5 * sum(es[128 * b:128 * (b + 1)].sum() for b, h in ACT_SET)
        corrQ = 0.5 * sum((es[128 * b:128 * (b + 1)] ** 2).sum() for b, h in ACT_SET)
        C = o[0] + 0.5 * 128 * len(ACT_SET)
        P = o[1] + o[2] + corrP
        Q = o[3] + o[4] + corrQ
        ew = e[(SLAB * c + np.arange(NW)) % N]
        S += float((coef * (C * ew * ew + Q - 2.0 * P * ew)).sum())
        n += float((coef * C).sum())
        bce_sum += float(o[5, 0:128].sum())

    bce = bce_sum / N
    if use_BCE_loss_only:
        return np.array(bce, dtype=np.float32)
    loss = bce + (MSE_WEIGHT * S / max(n, 1.0) if n > 0 else 0.0)
    return np.array(loss, dtype=np.float32)


# revision 21
# speedup vs baseline: 1.2477x; 1.2477x over previous
"""Trainium2 Bass kernel for AllExamplesPairwiseMSELossAndBCEWithLogitsLoss.

loss = BCE_with_logits(pred, psi) + 10 * sum_valid((e_i - e_j)^2) / n_valid
where e = pred - logit(psi) and valid pairs satisfy |psi_i - psi_j| >= 0.05.

Uses the identity pred_diff - true_diff = e_i - e_j and the decomposition
  S = sum_{r,j} M_rj (e_r - e_j)^2 = sum_j [C_j e_j^2 + Q_j - 2 e_j P_j]
with (C,P,Q)_j = sum_r M_rj * (1, e_r, e_r^2), so the only O(N^2) work is
mask generation (VectorEngine dual-op tensor_scalar, fp32 compare -> bf16
mask) and the row contraction (TensorEngine matmul, bf16 masks streamed
against a tiny stationary [128,5] operand (1, e_hi, e_lo, e2_hi, e2_lo) --
hi/lo bf16 splitting keeps ~1e-5 precision at 1 cycle/column).

Data-parallel over 8 NeuronCores: core c owns rows [1024c, 1024c+1024).
Each core emits C/P/Q [5, 8192] + per-partition BCE partials; the host
unshards by summing per-core partials into the final scalar.
"""

import numpy as np

import concourse.bass as bass
import concourse.bacc as bacc
import concourse.tile as tile
import concourse.mybir as mybir
from concourse.bass_utils import run_bass_kernel_spmd

F32 = mybir.dt.float32
BF16 = mybir.dt.bfloat16
ALU = mybir.AluOpType
ACT = mybir.ActivationFunctionType

N = 8192
NCORES = 8
SLAB = N // NCORES          # 1024 rows per core
RB = SLAB // 128            # 8 row blocks of 128
W = 512                     # column window (one PSUM bank of fp32)
# circulant column window: core c covers columns (1024c + k) mod N, k < NW.
# Each unordered pair lands in exactly one core's window, except the own-slab
# (k < 1024) and antipode (k >= 4096) regions which pair up across cores --
# region coefficients (1, 2, 1) in the host reduction count every ordered
# pair exactly once.
NW = 5 * SLAB               # 5120 columns per core
CHUNKS = (2560, 2560)
CHUNK = 2560                # max DVE mask tile width
THRESH = 0.05
EPS = 1e-7
MSE_WEIGHT = 10.0

# (block, half) mask units computed on the ScalarEngine via Sign instead of
# the VectorEngine's is_ge/is_le. For those, mask_half = (sign +- 1)/2, which
# is folded in as (+-w5/2)^T sign plus a per-column-constant correction of
# 0.5 * sum_{r in block} w5[r] that the host adds back (see kernel()).
ACT_SET = ((0, 1), (2, 1), (4, 1), (6, 1), (7, 0))

_compiled_nc = None


def build_nc():
    nc = bacc.Bacc("TRN2", target_bir_lowering=False, debug=False, num_devices=NCORES)

    cw_d = nc.dram_tensor("cw", [1, NW], F32, kind="ExternalInput")      # psi, column order
    cs_d = nc.dram_tensor("cs", [128, RB], F32, kind="ExternalInput")   # psi slab, [p, b] = psi[128b + p]
    ps_d = nc.dram_tensor("ps", [128, RB], F32, kind="ExternalInput")   # pred slab, same layout
    out_d = nc.dram_tensor("out", [6, NW], F32, kind="ExternalOutput")

    with tile.TileContext(nc) as tc:
        with (
            tc.tile_pool(name="singles", bufs=1) as singles,
            tc.tile_pool(name="small", bufs=1) as small,
            tc.tile_pool(name="mdve", bufs=14) as mdve,
            tc.tile_pool(name="mact", bufs=7) as mact,
            tc.tile_pool(name="drains", bufs=2) as drains,
            tc.tile_pool(name="psum", bufs=1, space="PSUM") as psum_pool,
        ):
            # slab inputs first (tiny, and the whole prep chain hangs off them),
            # then the column broadcasts fill cj while prep runs
            cs = singles.tile([128, RB], F32)
            ps = singles.tile([128, RB], F32)
            nc.sync.dma_start(cs[:], cs_d[:])
            nc.sync.dma_start(ps[:], ps_d[:])
            cj = singles.tile([128, NW], F32)
            for w in range(NW // W):
                nc.sync.dma_start(
                    cj[:, W * w:W * (w + 1)],
                    cw_d[0:1, W * w:W * (w + 1)].partition_broadcast(128),
                )

            cc = small.tile([128, RB], F32, tag="cc")
            nc.vector.tensor_scalar(
                out=cc[:], in0=cs[:], scalar1=EPS, scalar2=1.0 - EPS,
                op0=ALU.max, op1=ALU.min,
            )
            lnp = small.tile([128, RB], F32, tag="lnp")
            nc.scalar.activation(out=lnp[:], in_=cc[:], func=ACT.Ln)
            ln1mp = small.tile([128, RB], F32, tag="ln1mp")
            nc.scalar.activation(out=ln1mp[:], in_=cc[:], func=ACT.Ln, scale=-1.0, bias=1.0)
            blog = small.tile([128, RB], F32, tag="blog")
            nc.vector.tensor_sub(blog[:], lnp[:], ln1mp[:])
            e_sl = singles.tile([128, RB], F32)
            nc.vector.tensor_sub(e_sl[:], ps[:], blog[:])
            esq_sl = singles.tile([128, RB], F32)
            nc.vector.tensor_mul(esq_sl[:], e_sl[:], e_sl[:])

            # hi/lo bf16 split of e and e^2
            w5 = singles.tile([128, RB, 5], BF16)
            nc.vector.memset(w5[:, :, 0], 1.0)
            nc.vector.tensor_copy(w5[:, :, 1], e_sl[:])          # e_hi (bf16 round)
            e_hi = small.tile([128, RB], F32, tag="ehi")
            nc.vector.tensor_copy(e_hi[:], w5[:, :, 1])          # upcast back
            e_lo = small.tile([128, RB], F32, tag="elo")
            nc.vector.tensor_sub(e_lo[:], e_sl[:], e_hi[:])
            nc.vector.tensor_copy(w5[:, :, 2], e_lo[:])
            nc.vector.tensor_copy(w5[:, :, 3], esq_sl[:])
            q_hi = small.tile([128, RB], F32, tag="qhi")
            nc.vector.tensor_copy(q_hi[:], w5[:, :, 3])
            q_lo = small.tile([128, RB], F32, tag="qlo")
            nc.vector.tensor_sub(q_lo[:], esq_sl[:], q_hi[:])
            nc.vector.tensor_copy(w5[:, :, 4], q_lo[:])

            # half-scaled weights for the Sign-mask units, and their biases
            w5h = singles.tile([128, RB, 5], BF16)
            nc.scalar.activation(out=w5h[:], in_=w5[:], func=ACT.Copy, scale=0.5)
            w5hn = singles.tile([128, RB, 5], BF16)
            nc.scalar.activation(out=w5hn[:], in_=w5[:], func=ACT.Copy, scale=-0.5)
            bp = singles.tile([128, RB], F32)   # -(cs + t), bias for h=0 sign masks
            nc.vector.tensor_scalar(out=bp[:], in0=cs[:], scalar1=-1.0, scalar2=-THRESH,
                                    op0=ALU.mult, op1=ALU.add)
            bm = singles.tile([128, RB], F32)   # -(cs - t), bias for h=1 sign masks
            nc.vector.tensor_scalar(out=bm[:], in0=cs[:], scalar1=-1.0, scalar2=THRESH,
                                    op0=ALU.mult, op1=ALU.add)

            # BCE partial: max(x,0) - x*y + softplus(-|x|), summed along free dim
            relux = small.tile([128, RB], F32, tag="relux")
            nc.vector.tensor_scalar_max(relux[:], ps[:], 0.0)
            xy = small.tile([128, RB], F32, tag="xy")
            nc.vector.tensor_mul(xy[:], ps[:], cs[:])
            t1 = small.tile([128, RB], F32, tag="t1")
            nc.vector.tensor_sub(t1[:], relux[:], xy[:])
            ax = small.tile([128, RB], F32, tag="ax")
            nc.scalar.activation(out=ax[:], in_=ps[:], func=ACT.Abs)
            # softplus(-|x|) = ln(1 + exp(-|x|)); exp(-|x|) in (0.01, 1] here so
            # plain ln(1+t) is accurate (Softplus has no activation table on gen3)
            ex = small.tile([128, RB], F32, tag="ex")
            nc.scalar.activation(out=ex[:], in_=ax[:], func=ACT.Exp, scale=-1.0)
            sp = small.tile([128, RB], F32, tag="sp")
            nc.scalar.activation(out=sp[:], in_=ex[:], func=ACT.Ln, bias=1.0)
            tot = small.tile([128, RB], F32, tag="tot")
            nc.vector.tensor_add(tot[:], t1[:], sp[:])
            bce_red = singles.tile([128, 1], F32)
            nc.vector.tensor_reduce(bce_red[:], tot[:], axis=mybir.AxisListType.X, op=ALU.add)
            nc.sync.dma_start(out_d[5:6, 0:128], bce_red[:])

            # ---- main O(N^2) loop: masks on DVE + ACT(Sign), contraction on PE ----
            c0 = 0
            for chunk in CHUNKS:
                mts = []
                for b in range(RB):
                    for h in range(2):
                        if (b, h) in ACT_SET:
                            mt = mact.tile([128, CHUNK], BF16, tag="ms", name="mts")[:, 0:chunk]
                            nc.scalar.activation(
                                out=mt[:], in_=cj[:, c0:c0 + chunk], func=ACT.Sign,
                                bias=(bp if h == 0 else bm)[:, b:b + 1],
                            )
                            lhs = (w5h if h == 0 else w5hn)[:, b, :]
                        else:
                            mt = mdve.tile([128, CHUNK], BF16, tag="m", name="mtv")[:, 0:chunk]
                            nc.vector.tensor_scalar(
                                out=mt[:], in0=cj[:, c0:c0 + chunk],
                                scalar1=cs[:, b:b + 1],
                                scalar2=THRESH if h == 0 else -THRESH,
                                op0=ALU.subtract,
                                op1=ALU.is_ge if h == 0 else ALU.is_le,
                            )
                            lhs = w5[:, b, :]
                        mts.append((lhs, mt))
                # col-tiled contraction: window w runs in PE column group w%4 and
                # accumulates into its own PSUM bank (concurrent col-tiled matmuls
                # corrupt results when they share a bank; distinct banks are exact)
                ps_t = psum_pool.tile([128, CHUNK], F32, tag="ps", name="pst")[:, 0:chunk]
                for i, (lhs, mt) in enumerate(mts):
                    for w in range(chunk // W):
                        j = w % 4
                        nc.tensor.matmul(
                            ps_t[32 * j:32 * j + 5, W * w:W * (w + 1)],
                            lhs, mt[:, W * w:W * (w + 1)],
                            start=(i == 0), stop=(i == len(mts) - 1),
                            tile_position=(0, 32 * j),
                        )
                dr = drains.tile([128, CHUNK], F32, tag="drain", name="drt")[:, 0:chunk]
                nc.scalar.copy(dr[:], ps_t[:])
                for w in range(chunk // W):
                    j = w % 4
                    nc.sync.dma_start(
                        out_d[0:5, c0 + W * w:c0 + W * (w + 1)],
                        dr[32 * j:32 * j + 5, W * w:W * (w + 1)],
                    )
                c0 += chunk

    nc.compile()
    return nc


def _get_nc():
    global _compiled_nc
    if _compiled_nc is None:
        _compiled_nc = build_nc()
    return _compiled_nc


def _logit_np(p):
    p = np.clip(p.astype(np.float32), EPS, np.float32(1.0 - EPS))
    return (np.log(p) - np.log1p(-p)).astype(np.float32)


def run(pred, psi, trace=False):
    """Run the device kernel; returns BassKernelResults."""
    nc = _get_nc()
    in_maps = []
    for c in range(NCORES):
        sl = slice(SLAB * c, SLAB * (c + 1))
        cols = (SLAB * c + np.arange(NW)) % N
        in_maps.append({
            "cw": np.ascontiguousarray(psi[cols].reshape(1, NW)),
            "cs": np.ascontiguousarray(psi[sl].reshape(RB, 128).T),
            "ps": np.ascontiguousarray(pred[sl].reshape(RB, 128).T),
        })
    res = run_bass_kernel_spmd(nc, in_maps, core_ids=list(range(NCORES)), trace=trace)
    return res


def kernel(pred_psi_val, psi_val, use_BCE_loss_only=0):
    pred = np.asarray(pred_psi_val, dtype=np.float32).reshape(N)
    psi = np.asarray(psi_val, dtype=np.float32).reshape(N)

    res = run(pred, psi)

    e = (pred - _logit_np(psi)).astype(np.float64)
    coef = np.ones(NW)
    coef[SLAB:4 * SLAB] = 2.0
    S = 0.0
    n = 0.0
    bce_sum = 0.0
    for c in range(NCORES):
        o = res.results[c]["out"].astype(np.float64)
        es = e[SLAB * c:SLAB * (c + 1)]
        corrP = 0.5 * sum(es[128 * b:128 * (b + 1)].sum() for b, h in ACT_SET)
        corrQ = 0.5 * sum((es[128 * b:128 * (b + 1)] ** 2).sum() for b, h in ACT_SET)
        C = o[0] + 0.5 * 128 * len(ACT_SET)
        P = o[1] + o[2] + corrP
        Q = o[3] + o[4] + corrQ
        ew = e[(SLAB * c + np.arange(NW)) % N]
        S += float((coef * (C * ew * ew + Q - 2.0 * P * ew)).sum())
        n += float((coef * C).sum())
        bce_sum += float(o[5, 0:128].sum())

    bce = bce_sum / N
    if use_BCE_loss_only:
        return np.array(bce, dtype=np.float32)
    loss = bce + (MSE_WEIGHT * S / max(n, 1.0) if n > 0 else 0.0)
    return np.array(loss, dtype=np.float32)
